# revision 1
# baseline (speedup 1.0000x reference)
"""Distributed Trainium2 (Bass/Tile) kernel for single-head latent attention.

Reference computation (B=4, S=4096, D=1024, DL=64):
    qkv = x @ Wd + bd; q,k,v = split(qkv)
    logits = (q @ k^T) / sqrt(DL) / TEMP, key-masked
    out = softmax(logits) @ v @ Wu + bu

Sharding: data-parallel over (batch, seq-half) -> 8 shards of 2048 query rows.
Each core recomputes K/V for its batch's keys from x (no collectives).

Key tricks:
  - Host-side mask compaction: only unmasked rows (~2040 of 4096, capped at
    K_CAP=2176) are gathered as keys, cutting the S^2 attention work ~2x.
    Pad slots get exp-bias -1e30 -> zero weight.
  - All layouts chosen so no activation transposes are needed (except 17
    tiny PE transposes for V): projection emits qT/kT/vT directly.
  - Softmax without row-max: scaled logits are bounded (~±95), shifted by
    -40 in the exp bias, so exp/sums stay finite in fp32 and the flash
    accumulation over key chunks is plain PSUM accumulation.
  - PV matmul lhsT is [ones | v] [128, 65]: row 0 of the accumulator is Z,
    rows 1:65 are ctxU. After normalizing by broadcast(1/Z) row 0 becomes
    exactly 1.0, and the up-projection rhs [bu; Wu] folds in the bias.
  - dtypes: x/Wd/q/k/Wu fp16 (bf16's 8-bit mantissa fails: exp amplifies
    logit error to ~1.2e-2; fp16's 10 bits keep it ~2.5e-3), exp/v bf16
    (exp values overflow fp16), v-transposes float32r, out f16.
  - Attention runs as two q-passes (cols 0:1024, 1024:2048) so PSUM fits
    double-buffered [128,1024] logits tiles + the ctx/Z accumulator; MM2
    for chunk c is emitted after MM1 of chunk c+2 so the in-order PE queue
    never stalls on exp; pass A's ctx copies, normalization, and first
    up-projection tiles ride inside pass B's ACT-paced stream.
  - The PE HAM clock-gate drops to 1.2 GHz unless the PE is strictly
    saturated; WAW-chained dummy matmuls into an unread PSUM bank fill
    every DMA/ACT stall so the array holds 2.4 GHz (~25% net win).
"""

import sys

if "/opt/trn_rl_repo" not in sys.path:
    sys.path.insert(0, "/opt/trn_rl_repo")

import numpy as np

from concourse import bacc, tile
from concourse import mybir
from concourse.masks import make_identity

F32 = mybir.dt.float32
F32R = mybir.dt.float32r
BF16 = mybir.dt.bfloat16
F16 = mybir.dt.float16

B, S, D, DL = 4, 4096, 1024, 64
N_CORES = 8
S_LOC = S // 2          # 2048 query rows per core
SR = 512
JC = 128                # key chunk
NJK = 17                # compacted key chunks
K_CAP = NJK * JC        # 2176 >= max unmasked keys per batch (~2076 @ +3σ
                        # above the Binomial(4096,1/2) mean of 2048)
QH = 1024               # logits/exp q-tile width (one attention pass)
SCALE = 1.25            # 1/sqrt(64)/0.1
LOGIT_SHIFT = -40.0
MASKED_BIAS = -1e30

_CACHE = {}


def build_graph():
    """Core-agnostic Bacc graph; each core's inputs are pre-sliced host-side
    (local query half + compacted keys of its batch, in d-chunk slabs)."""
    nc = bacc.Bacc("TRN2", target_bir_lowering=False, debug=False,
                   num_devices=N_CORES)

    xT_d = nc.dram_tensor("xT", [8, 128, S_LOC], F16, kind="ExternalInput").ap()
    xk_d = nc.dram_tensor("xkT", [8, 128, K_CAP], F16, kind="ExternalInput").ap()
    wd_d = nc.dram_tensor("Wd", [128, 8 * 192], F16, kind="ExternalInput").ap()
    wub_d = nc.dram_tensor("Wub", [DL + 1, D], F16, kind="ExternalInput").ap()
    bdq_d = nc.dram_tensor("bd_q", [64, 1], F32, kind="ExternalInput").ap()
    bdkv_d = nc.dram_tensor("bd_kv", [128, 1], F32, kind="ExternalInput").ap()
    mb_d = nc.dram_tensor("maskbias", [128, NJK], F32, kind="ExternalInput").ap()
    out_d = nc.dram_tensor("out", [S_LOC, D], F16, kind="ExternalOutput").ap()

    kv_ranges = []                      # (col0, width) covering K_CAP
    c0 = 0
    while c0 < K_CAP:
        w = min(SR, K_CAP - c0)
        kv_ranges.append((c0, w))
        c0 += w

    with tile.TileContext(nc) as tc, nc.allow_low_precision(
            reason="float32r/bf16/f16 tiles feed full-rate PE matmuls; "
                   "~10-bit mantissas are far inside the 2e-2 error budget"):
        with (
            tc.tile_pool(name="consts", bufs=1) as consts,
            tc.tile_pool(name="acts", bufs=1) as acts,
            tc.tile_pool(name="ep", bufs=6) as ep,
        ):
            # ---- constants -------------------------------------------------
            wd_s = consts.tile([128, 8 * 192], F16)
            nc.sync.dma_start(out=wd_s[:], in_=wd_d[:])
            wub_s = consts.tile([DL + 1, D], F16)
            nc.gpsimd.dma_start(out=wub_s[:], in_=wub_d[:])
            bdq_s = consts.tile([64, 1], F32)
            nc.gpsimd.dma_start(out=bdq_s[:], in_=bdq_d[:])
            bdkv_s = consts.tile([128, 1], F32)
            nc.gpsimd.dma_start(out=bdkv_s[:], in_=bdkv_d[:])
            mb_s = consts.tile([128, NJK], F32)
            nc.gpsimd.dma_start(out=mb_s[:], in_=mb_d[:])
            # preload the exp ACT table set early so the ~2.7us table-load
            # stall doesn't hit the PE pipeline at attention start
            act_warm = consts.tile([128, NJK], F32)
            nc.scalar.activation(act_warm[:], mb_s[:],
                                 mybir.ActivationFunctionType.Exp)
            # identity at partitions 64:128 (v rows live there)
            ident2f = consts.tile([128, 64], F32)
            nc.vector.memset(ident2f[:], 0.0)
            make_identity(nc, ident2f[64:128, :], nomemset=True)
            ident2 = consts.tile([128, 64], F32R)
            nc.vector.tensor_copy(ident2[:], ident2f[:])
            ones_colf = consts.tile([1, 128], F32)
            nc.vector.memset(ones_colf[:], 1.0)
            ones_col = consts.tile([1, 128], F32R)
            nc.vector.tensor_copy(ones_col[:], ones_colf[:])

            # ---- x slabs + activations (SBUF-resident) ---------------------
            xq_sb = acts.tile([128, 8 * S_LOC], F16)
            xk_sb = acts.tile([128, 8 * K_CAP], F16)
            for k in range(8):
                nc.sync.dma_start(out=xk_sb[:, k * K_CAP:(k + 1) * K_CAP],
                                  in_=xk_d[k])
            for k in range(8):
                nc.scalar.dma_start(out=xq_sb[:, k * S_LOC:(k + 1) * S_LOC],
                                    in_=xT_d[k])
            qT_s = acts.tile([64, S_LOC], F16)
            kT_s = acts.tile([64, K_CAP], F16)
            # vT at partitions 64:128 so the fused k|v psum copies shift-free
            vT_hi = acts.tile([128, K_CAP], F32R)
            # PV stationary per key chunk: col 0 = ones, cols 1:65 = v
            v_aug = acts.tile([128, NJK * 65], BF16)
            nc.vector.memset(v_aug[:], 1.0)
            ctxu_s = acts.tile([DL + 1, S_LOC], F32R)
            rzb_s = acts.tile([DL + 1, S_LOC], F32)
            rzb_scr = acts.tile([DL + 1, S_LOC], F32)
            ctxn_s = acts.tile([DL + 1, S_LOC], F16)

            # dummy-warmup matmuls: keep the PE HAM activity monitor busy
            # through DMA/ACT stalls so the clock stays at 2.4 GHz. Writes
            # an unread PSUM bank; WAW-chained so they fill in queue order.
            dwp_cm = tc.tile_pool(name="dw", bufs=1, space="PSUM")
            dwp = dwp_cm.__enter__()
            dummy_ps = dwp.tile([128, SR], F32, name="dummy_ps")

            def warm(n):
                for _ in range(n):
                    nc.tensor.matmul(dummy_ps[:], wd_s[:, 0:128],
                                     wd_s[:, 0:SR], start=True, stop=True)

            warm(24)    # cover the initial x-slab DMA wall

            # ---- phase 1: projections --------------------------------------
            with (
                tc.tile_pool(name="pp", bufs=3, space="PSUM") as pp,
                tc.tile_pool(name="pt", bufs=2, space="PSUM") as pt,
            ):
                def kv_range(r, c0, w):
                    # fused k|v: Wd cols 64:192 -> psum rows 0:64 k, 64:128 v
                    ps_kv = pp.tile([128, SR], F32, tag="p", name=f"pskv{r}")
                    for k in range(8):
                        nc.tensor.matmul(
                            ps_kv[:, 0:w], wd_s[:, k * 192 + 64:(k + 1) * 192],
                            xk_sb[:, k * K_CAP + c0:k * K_CAP + c0 + w],
                            start=(k == 0), stop=(k == 7))
                    nc.vector.tensor_scalar_add(kT_s[:, c0:c0 + w],
                                                ps_kv[0:64, 0:w],
                                                bdkv_s[0:64, :])
                    nc.vector.tensor_scalar_add(vT_hi[64:128, c0:c0 + w],
                                                ps_kv[64:128, 0:w],
                                                bdkv_s[64:128, :])
                    # transpose this range's v chunks into v_aug
                    for c in range(c0 // JC, (c0 + w) // JC):
                        vt_ps = pt.tile([128, 64], F32R, tag="t",
                                        name=f"vt{c}")
                        nc.tensor.transpose(vt_ps[:],
                                            vT_hi[64:128, c * JC:(c + 1) * JC],
                                            ident2[64:128, :])
                        nc.vector.tensor_copy(
                            v_aug[:, c * 65 + 1:(c + 1) * 65], vt_ps[:])

                def q_range(r):
                    ps_q = pp.tile([64, SR], F32, tag="p", name=f"psq{r}")
                    for k in range(8):
                        nc.tensor.matmul(
                            ps_q[:], wd_s[:, k * 192:k * 192 + 64],
                            xq_sb[:, k * S_LOC + r * SR:
                                  k * S_LOC + (r + 1) * SR],
                            start=(k == 0), stop=(k == 7))
                    nc.vector.tensor_scalar_add(
                        qT_s[:, r * SR:(r + 1) * SR], ps_q[:], bdq_s[:])

                for r, (c0, w) in enumerate(kv_ranges):
                    kv_range(r, c0, w)
                    warm(2)
                for r in range(S_LOC // SR):
                    q_range(r)
                    warm(2)

            warm(12)    # bridge the proj->attention pool transition

            # ---- phase 2+3: attention (two q-passes) + up-projection -------
            # MM2 for chunk c is emitted after MM1 of chunk c+2 so the
            # in-order PE queue never stalls waiting for exp(c); pass A's
            # up-projection tiles ride inside pass B's ACT-paced stream.
            with (
                tc.tile_pool(name="pl", bufs=2, space="PSUM") as pl,
                tc.tile_pool(name="pc", bufs=1, space="PSUM") as pc,
                tc.tile_pool(name="po", bufs=1, space="PSUM") as po,
                tc.tile_pool(name="ob", bufs=3) as ob,
            ):
                def up_tile(st, allow_scalar=False):
                    osb = ob.tile([128, D], F16, tag="ot", name=f"osb{st}")
                    for s2 in range(2):
                        up = po.tile([128, SR], F32, tag="o", name=f"up{st}_{s2}")
                        nc.tensor.matmul(
                            up[:], ctxn_s[:, st * 128:(st + 1) * 128],
                            wub_s[:, s2 * SR:(s2 + 1) * SR],
                            start=True, stop=True)
                        if allow_scalar and st % 2 == 1:
                            nc.scalar.copy(osb[:, s2 * SR:(s2 + 1) * SR], up[:])
                        else:
                            nc.vector.tensor_copy(
                                osb[:, s2 * SR:(s2 + 1) * SR], up[:])
                    nc.sync.dma_start(out=out_d[st * 128:(st + 1) * 128, :],
                                      in_=osb[:])

                def epilogue(pas):
                    q0 = pas * QH
                    for s2 in range(2):
                        sl = slice(q0 + s2 * SR, q0 + (s2 + 1) * SR)
                        zb = pl.tile([DL + 1, SR], F32, tag="l",
                                     name=f"zb{pas}_{s2}")
                        nc.tensor.matmul(zb[:], ones_col[:, 0:DL + 1],
                                         ctxu_s[0:1, sl], start=True, stop=True)
                        nc.vector.reciprocal_approx_accurate(
                            rzb_s[:, sl], zb[:], rzb_scr[:, sl])
                    sl = slice(q0, q0 + QH)
                    nc.vector.tensor_mul(ctxn_s[:, sl], ctxu_s[:, sl],
                                         rzb_s[:, sl])

                ctx_tiles = {}
                for pas in range(2):
                    q0 = pas * QH
                    ctx_ps = pc.tile([DL + 1, QH], F32, tag="c",
                                     name=f"ctx{pas}")
                    ctx_tiles[pas] = ctx_ps
                    exs = {}

                    def mm2(c):
                        for s2 in range(2):
                            nc.tensor.matmul(
                                ctx_ps[:, s2 * SR:(s2 + 1) * SR],
                                v_aug[:, c * 65:(c + 1) * 65],
                                exs[c][:, s2 * SR:(s2 + 1) * SR],
                                start=(c == 0), stop=(c == NJK - 1))

                    for c in range(NJK):
                        if pas == 0:
                            warm(2 if c % 2 == 0 else 1)
                        elif not (c >= 4 and c % 2 == 0):
                            warm(1)
                        lg = pl.tile([128, QH], F32, tag="l",
                                     name=f"lg{pas}_{c}")
                        for s2 in range(2):
                            nc.tensor.matmul(
                                lg[:, s2 * SR:(s2 + 1) * SR],
                                kT_s[:, c * JC:(c + 1) * JC],
                                qT_s[:, q0 + s2 * SR:q0 + (s2 + 1) * SR],
                                start=True, stop=True)
                        ex = ep.tile([128, QH], BF16, tag="e",
                                     name=f"ex{pas}_{c}")
                        nc.scalar.activation(
                            ex[:], lg[:], mybir.ActivationFunctionType.Exp,
                            bias=mb_s[:, c:c + 1], scale=SCALE)
                        exs[c] = ex
                        if c >= 3:
                            mm2(c - 3)
                        if pas == 1:
                            # pass A copies/epilogue/up-proj ride in pass B
                            if c == 1:
                                for s2 in range(2):
                                    sl = slice(s2 * SR, (s2 + 1) * SR)
                                    nc.vector.tensor_copy(
                                        ctxu_s[:, sl],
                                        ctx_tiles[0][:, s2 * SR:(s2 + 1) * SR])
                            if c == 2:
                                epilogue(0)
                            if c >= 4 and c % 2 == 0:
                                up_tile((c - 4) // 2)
                    mm2(NJK - 3)
                    mm2(NJK - 2)
                    mm2(NJK - 1)
                    if pas == 1:
                        for s2 in range(2):
                            sl = slice(q0 + s2 * SR, q0 + (s2 + 1) * SR)
                            nc.vector.tensor_copy(
                                ctxu_s[:, sl], ctx_ps[:, s2 * SR:(s2 + 1) * SR])
                epilogue(1)
                warm(6)

            dwp_cm.__exit__(None, None, None)

            # ---- tail: remaining up-projection tiles, deep-pipelined -------
            with (
                tc.tile_pool(name="po2", bufs=3, space="PSUM") as po2,
                tc.tile_pool(name="ob2", bufs=3) as ob2,
            ):
                for st in range(7, 16):
                    osb = ob2.tile([128, D], F16, tag="o2", name=f"osb2_{st}")
                    for s2 in range(2):
                        up = po2.tile([128, SR], F32, tag="u",
                                      name=f"upt{st}_{s2}")
                        nc.tensor.matmul(
                            up[:], ctxn_s[:, st * 128:(st + 1) * 128],
                            wub_s[:, s2 * SR:(s2 + 1) * SR],
                            start=True, stop=True)
                        if s2 == 0:
                            nc.vector.tensor_copy(
                                osb[:, s2 * SR:(s2 + 1) * SR], up[:])
                        else:
                            nc.scalar.copy(osb[:, s2 * SR:(s2 + 1) * SR],
                                           up[:])
                    nc.sync.dma_start(out=out_d[st * 128:(st + 1) * 128, :],
                                      in_=osb[:])

    nc.compile()
    return nc


def get_graph():
    if "graph" not in _CACHE:
        _CACHE["graph"] = build_graph()
    return _CACHE["graph"]


def make_in_maps(x, attention_mask, Wd, bd, Wu, bu):
    wub = np.ascontiguousarray(
        np.concatenate([bu[None, :], Wu], axis=0).astype(np.float16))
    wd_c = np.ascontiguousarray(
        Wd.astype(np.float16).reshape(8, 128, 192).transpose(1, 0, 2)
        .reshape(128, 8 * 192))
    bd_q = np.ascontiguousarray(bd[0:64].reshape(64, 1).astype(np.float32))
    bd_kv = np.ascontiguousarray(bd[64:192].reshape(128, 1).astype(np.float32))
    per_batch = []
    for b in range(B):
        idx = np.nonzero(attention_mask[b])[0]
        n = len(idx)
        assert n <= K_CAP, f"unmasked key count {n} exceeds K_CAP={K_CAP}"
        idxp = np.concatenate([idx, np.zeros(K_CAP - n, np.int64)])
        xkT = np.ascontiguousarray(
            x[b][idxp].T.astype(np.float16).reshape(8, 128, K_CAP))
        mb = np.full(K_CAP, MASKED_BIAS, np.float32)
        mb[:n] = LOGIT_SHIFT
        per_batch.append((xkT, np.ascontiguousarray(mb.reshape(NJK, 128).T)))
    in_maps = []
    for c in range(N_CORES):
        b, h = c // 2, c % 2
        xkT, mb = per_batch[b]
        xT = np.ascontiguousarray(
            x[b, h * S_LOC:(h + 1) * S_LOC].T.astype(np.float16)
            .reshape(8, 128, S_LOC))
        in_maps.append({
            "xT": xT,
            "xkT": xkT,
            "Wd": wd_c,
            "Wub": wub,
            "bd_q": bd_q,
            "bd_kv": bd_kv,
            "maskbias": mb,
        })
    return in_maps


def kernel(x, attention_mask, Wd, bd, Wu, bu):
    from concourse import bass_utils

    x = np.asarray(x, dtype=np.float32)
    attention_mask = np.asarray(attention_mask)
    Wd = np.asarray(Wd, dtype=np.float32)
    bd = np.asarray(bd, dtype=np.float32)
    Wu = np.asarray(Wu, dtype=np.float32)
    bu = np.asarray(bu, dtype=np.float32)

    nc = get_graph()
    in_maps = make_in_maps(x, attention_mask, Wd, bd, Wu, bu)
    res = bass_utils.run_bass_kernel_spmd(nc, in_maps, list(range(N_CORES)))
    out = np.empty((B, S, D), dtype=np.float32)
    for c in range(N_CORES):
        b, h = c // 2, c % 2
        out[b, h * S_LOC:(h + 1) * S_LOC, :] = \
            res.results[c]["out"].astype(np.float32)
    return out



# revision 16
# speedup vs baseline: 1.0056x; 1.0056x over previous
"""Distributed Trainium2 (Bass/Tile) kernel for single-head latent attention.

Reference computation (B=4, S=4096, D=1024, DL=64):
    qkv = x @ Wd + bd; q,k,v = split(qkv)
    logits = (q @ k^T) / sqrt(DL) / TEMP, key-masked
    out = softmax(logits) @ v @ Wu + bu

Sharding: data-parallel over (batch, seq-half) -> 8 shards of 2048 query rows.
Each core recomputes K/V for its batch's keys from x (no collectives).

v2 design (vs the 124us baseline):
  - Fine-grained range-major input DMA on 3 queues; projections chase the
    arriving data and attention pair 0 starts at ~8us instead of ~35us.
  - MM1 row-tiled (contraction DL=64): chunk pairs run concurrently on the
    two 64-row halves of the PE array (tile_position (0,0)/(64,0)).  kT uses
    a parity layout (even kv-ranges' k in partitions 0:64, odd ranges' in
    64:128) produced with per-range-swapped [k|v]/[v|k] projection weights,
    so no cross-partition moves are needed anywhere.  q is duplicated into
    both halves for free by duplicating Wd's q columns (M=64 -> M=128).
  - Postponed normalization: out = (ctxU @ Wu) * (1/Z) + bu.  1/Z is applied
    per-query at PSUM evacuation (scalar_tensor_tensor), ctxU flows bf16
    (unnormalized ctx spans e^+-55 -> overflows fp16 but not bf16).  Z is
    transposed to query-major via 16 tiny PE transposes + one reciprocal.
  - Scalar engine does exp ONLY (34 x [128,1024] ACTIVATEs ~ 34us is the
    serial floor); every copy/evac lives on vector/gpsimd.
  - Up-projection row-tiled too: ctxU duplicated into partitions 64:128 by
    an SBUF->SBUF DMA; odd st tiles run on T8 with Wu's duplicate rows.
  - Dummy matmuls only bridge the initial DMA ramp (HAM clock warm-up).
"""

import sys

if "/opt/trn_rl_repo" not in sys.path:
    sys.path.insert(0, "/opt/trn_rl_repo")

import numpy as np

from concourse import bacc, tile
from concourse import mybir
from concourse.masks import make_identity

F32 = mybir.dt.float32
F32R = mybir.dt.float32r
BF16 = mybir.dt.bfloat16
F16 = mybir.dt.float16

B, S, D, DL = 4, 4096, 1024, 64
N_CORES = 8
S_LOC = S // 2          # 2048 query rows per core
SR = 512
JC = 128                # key chunk
NJK = 17                # compacted key chunks
K_CAP = NJK * JC        # 2176 >= max unmasked keys per batch
QH = 1024               # one attention pass = 1024 query columns
SCALE = 1.25            # 1/sqrt(64)/0.1
LOGIT_SHIFT = -40.0
MASKED_BIAS = -1e30
UP_PACKED = True        # odd st tiles on T8 (needs ctxU dup DMA)

# kv ranges: (col0, width, parity).  Even ranges project with [k|v] weights
# (k -> psum rows 0:64), odd with [v|k] (k -> rows 64:128), so the k-half
# always evacuates same-partition into its kT2 half.
KV_RANGES = [(0, 512, 0), (512, 512, 1), (1024, 512, 0), (1536, 512, 1),
             (2048, 128, 0)]
# chunk -> kT2 half/block: top (rows 0:64) = ranges 0,2,4; bottom = 1,3
TOP_CHUNKS = [0, 1, 2, 3, 8, 9, 10, 11, 16]
BOT_CHUNKS = [4, 5, 6, 7, 12, 13, 14, 15]
PAIRS = list(zip(TOP_CHUNKS, BOT_CHUNKS))   # 8 concurrent pairs
SOLO = 16                                   # odd chunk count: top block 8

_CACHE = {}


def _chunk_block(c):
    """kT2 (half, block) for chunk c."""
    if c in TOP_CHUNKS:
        return 0, TOP_CHUNKS.index(c)
    return 1, BOT_CHUNKS.index(c)


def build_graph():
    nc = bacc.Bacc("TRN2", target_bir_lowering=False, debug=False,
                   num_devices=N_CORES)

    xT_d = nc.dram_tensor("xT", [8, 128, S_LOC], F16, kind="ExternalInput").ap()
    xk_d = nc.dram_tensor("xkT", [8, 128, K_CAP], F16, kind="ExternalInput").ap()
    wd_d = nc.dram_tensor("Wd2", [128, 8 * 384], F16, kind="ExternalInput").ap()
    wu_d = nc.dram_tensor("Wu2", [128, D], BF16, kind="ExternalInput").ap()
    bu_d = nc.dram_tensor("bu_row", [1, D], F32, kind="ExternalInput").ap()
    bdq_d = nc.dram_tensor("bd_q2", [128, 1], F32, kind="ExternalInput").ap()
    bdkv_d = nc.dram_tensor("bd_kv2", [128, 2], F32, kind="ExternalInput").ap()
    mb_d = nc.dram_tensor("maskbias", [128, NJK], F32, kind="ExternalInput").ap()
    out_d = nc.dram_tensor("out", [S_LOC, D], F16, kind="ExternalOutput").ap()

    with tile.TileContext(nc) as tc, nc.allow_low_precision(
            reason="bf16/f16 tiles feed full-rate PE matmuls; ~10-bit "
                   "mantissas are far inside the 2e-2 error budget"):
        with (
            tc.tile_pool(name="consts", bufs=1) as consts,
            tc.tile_pool(name="acts", bufs=1) as acts,
            tc.tile_pool(name="ep", bufs=4) as ep,
            tc.tile_pool(name="ob", bufs=3) as ob,
            tc.tile_pool(name="PL", bufs=2, space="PSUM") as PL,
            tc.tile_pool(name="PC", bufs=1, space="PSUM") as PC,
            tc.tile_pool(name="PP", bufs=1, space="PSUM") as PP,
            tc.tile_pool(name="PT", bufs=1, space="PSUM") as PT,
        ):
            # ---- tiny consts built on-device (no DMA dependency) ----------
            seed = consts.tile([128, SR], F16)
            nc.vector.memset(seed[:], 0.25)
            # exp ACT table preload (~2.7us) during the DMA ramp
            actwarm = consts.tile([128, 32], F32)
            nc.scalar.activation(actwarm[:], seed[:, 0:32],
                                 mybir.ActivationFunctionType.Exp)
            identf = consts.tile([128, 64], F32)
            nc.vector.memset(identf[:], 0.0)
            make_identity(nc, identf[0:64, :], nomemset=True)
            make_identity(nc, identf[64:128, :], nomemset=True)
            ident = consts.tile([128, 64], F32R)
            nc.vector.tensor_copy(ident[:], identf[:])
            onesf = consts.tile([128, 4], F32)
            nc.vector.memset(onesf[:], 1.0)
            onesr = consts.tile([128, 4], F32R)
            nc.vector.tensor_copy(onesr[:], onesf[:])
            onesrow = consts.tile([1, 128], F32)
            nc.vector.memset(onesrow[:], 1.0)

            # ---- DMA'd consts (gpsimd queue, small-first) -----------------
            bdq_s = consts.tile([128, 1], F32)
            nc.gpsimd.dma_start(out=bdq_s[:], in_=bdq_d[:])
            bdkv_s = consts.tile([128, 2], F32)
            nc.gpsimd.dma_start(out=bdkv_s[:], in_=bdkv_d[:])
            mb_s = consts.tile([128, NJK], F32)
            nc.gpsimd.dma_start(out=mb_s[:], in_=mb_d[:])
            wd_s = consts.tile([128, 8 * 384], F16)
            for k in range(8):
                nc.gpsimd.dma_start(out=wd_s[:, k * 384:(k + 1) * 384],
                                    in_=wd_d[:, k * 384:(k + 1) * 384])
            wu_s = consts.tile([128, D], BF16)
            bur_s = consts.tile([1, D], F32)

            # ---- activation tiles -----------------------------------------
            xq_sb = acts.tile([128, 8 * S_LOC], F16)
            xk_sb = acts.tile([128, 8 * K_CAP], F16)
            qT2 = acts.tile([128, S_LOC], F16)       # q duplicated both halves
            kT2 = acts.tile([128, 9 * JC], F16)      # parity layout
            vTb = acts.tile([128, K_CAP], F32R)      # v staging (half by range)
            v_aug = acts.tile([128, NJK * 65], BF16)  # [v(64) | ones] per chunk
            nc.vector.memset(v_aug[:], 1.0)
            ctxu = acts.tile([128, S_LOC], BF16)     # rows 0:64 ctx, 64:128 dup
            zr = acts.tile([128, S_LOC], F32)        # Z row staging (row 64)
            rzbc = acts.tile([128, 16], F32)         # 1/Z, query-major, col=st
            bub_s = consts.tile([128, D], F32)       # bu broadcast to 128 rows

            # ---- input DMAs, priority order, 3 queues ---------------------
            # attention-start set first on every queue; vector's later
            # triggers are staged between prologue pieces so its evac work
            # isn't stuck behind a wall of DMA_DIRECT2D instructions.
            def xk_dma(eng, k, c0, w):
                eng.dma_start(out=xk_sb[:, k * K_CAP + c0:k * K_CAP + c0 + w],
                              in_=xk_d[k][:, c0:c0 + w])

            def xq_dma(eng, k, c0, w):
                eng.dma_start(out=xq_sb[:, k * S_LOC + c0:k * S_LOC + c0 + w],
                              in_=xT_d[k][:, c0:c0 + w])

            for k in range(4):
                xk_dma(nc.sync, k, 0, 512)
            for k in range(4, 8):
                xk_dma(nc.scalar, k, 0, 512)
            for k in range(2):
                xq_dma(nc.gpsimd, k, 0, 1024)
            for k in range(2, 5):
                xq_dma(nc.sync, k, 0, 1024)
            for k in range(5, 8):
                xq_dma(nc.scalar, k, 0, 1024)
            for k in range(4):
                xk_dma(nc.sync, k, 512, 512)
            for k in range(4, 8):
                xk_dma(nc.scalar, k, 512, 512)
            for k in range(4):
                xk_dma(nc.sync, k, 1024, K_CAP - 1024)
            # wu/bu only needed from pass 1 (~25us)
            nc.gpsimd.dma_start(out=wu_s[:], in_=wu_d[:])
            nc.gpsimd.dma_start(out=bur_s[:], in_=bu_d[:])
            for k in range(2):
                xq_dma(nc.gpsimd, k, 1024, 1024)
            for k in range(2, 5):
                xq_dma(nc.sync, k, 1024, 1024)

            # ---- helpers --------------------------------------------------
            ndum = [0]

            def warm(n):
                for _ in range(n):
                    dmy = PL.tile([128, QH], F32, tag="L",
                                  name=f"dmy{ndum[0]}")
                    ndum[0] += 1
                    nc.tensor.matmul(dmy[:, 0:SR], seed[:, 0:128],
                                     seed[:], start=True, stop=True)

            def q_range(r):
                ps_q = PP.tile([128, SR], F32, tag="p", name=f"psq{r}")
                for k in range(8):
                    nc.tensor.matmul(
                        ps_q[:], wd_s[:, k * 384:k * 384 + 128],
                        xq_sb[:, k * S_LOC + r * SR:k * S_LOC + (r + 1) * SR],
                        start=(k == 0), stop=(k == 7))
                nc.vector.tensor_scalar_add(qT2[:, r * SR:(r + 1) * SR],
                                            ps_q[:], bdq_s[:, 0:1])

            def kv_range(ri):
                c0, w, par = KV_RANGES[ri]
                ps_kv = PP.tile([128, SR], F32, tag="p", name=f"pskv{ri}")
                wcol = 128 if par == 0 else 256
                for k in range(8):
                    nc.tensor.matmul(
                        ps_kv[:, 0:w],
                        wd_s[:, k * 384 + wcol:k * 384 + wcol + 128],
                        xk_sb[:, k * K_CAP + c0:k * K_CAP + c0 + w],
                        start=(k == 0), stop=(k == 7))
                half, blk0 = _chunk_block(c0 // JC)
                kh = slice(0, 64) if half == 0 else slice(64, 128)
                vh = slice(64, 128) if half == 0 else slice(0, 64)
                nc.vector.tensor_scalar_add(
                    kT2[kh, blk0 * JC:blk0 * JC + w], ps_kv[kh, 0:w],
                    bdkv_s[kh, par:par + 1])
                nc.vector.tensor_scalar_add(
                    vTb[vh, c0:c0 + w], ps_kv[vh, 0:w],
                    bdkv_s[vh, par:par + 1])

            def v_trans(ri):
                c0, w, par = KV_RANGES[ri]
                vh = slice(64, 128) if par == 0 else slice(0, 64)
                idh = ident[64:128, :] if par == 0 else ident[0:64, :]
                nch = w // JC
                vt_ps = PT.tile([128, 256], F32R, tag="t", name=f"vt{ri}")
                for j in range(nch):
                    c = c0 // JC + j
                    nc.tensor.transpose(
                        vt_ps[:, j * 64:(j + 1) * 64],
                        vTb[vh, c * JC:(c + 1) * JC], idh)
                for j in range(nch):
                    c = c0 // JC + j
                    nc.vector.tensor_copy(v_aug[:, c * 65:c * 65 + 64],
                                          vt_ps[:, j * 64:(j + 1) * 64])

            def bu_bcast():
                # bu broadcast: [1,1024] -> [128,1024] via two K=1 matmuls
                for s2 in range(2):
                    bu_ps = PT.tile([128, SR], F32, tag="t", name=f"bups{s2}")
                    nc.tensor.matmul(bu_ps[:], onesrow[:, 0:128],
                                     bur_s[:, s2 * SR:(s2 + 1) * SR],
                                     start=True, stop=True)
                    nc.vector.tensor_copy(bub_s[:, s2 * SR:(s2 + 1) * SR],
                                          bu_ps[:])

            warm(3)

            # ================ main software pipeline =======================
            exs = {}            # chunk -> ex tile (per pass, overwritten)
            ctx_tiles = {}
            nmm2 = [0]

            def mm1_exp(pas, ce, co):
                q0 = pas * QH
                lg_e = PL.tile([128, QH], F32, tag="L", name=f"lge{pas}_{ce}")
                _, blk = _chunk_block(ce)
                for s2 in range(2):
                    nc.tensor.matmul(
                        lg_e[:, s2 * SR:(s2 + 1) * SR],
                        kT2[0:64, blk * JC:(blk + 1) * JC],
                        qT2[0:64, q0 + s2 * SR:q0 + (s2 + 1) * SR],
                        start=True, stop=True)
                lg_o = None
                if co is not None:
                    _, blko = _chunk_block(co)
                    lg_o = PL.tile([128, QH], F32, tag="L",
                                   name=f"lgo{pas}_{co}")
                    for s2 in range(2):
                        nc.tensor.matmul(
                            lg_o[:, s2 * SR:(s2 + 1) * SR],
                            kT2[64:128, blko * JC:(blko + 1) * JC],
                            qT2[64:128, q0 + s2 * SR:q0 + (s2 + 1) * SR],
                            start=True, stop=True)
                ex_e = ep.tile([128, QH], BF16, tag="e", name=f"exe{pas}_{ce}")
                nc.scalar.activation(ex_e[:], lg_e[:],
                                     mybir.ActivationFunctionType.Exp,
                                     bias=mb_s[:, ce:ce + 1], scale=SCALE)
                exs[ce] = ex_e
                if co is not None:
                    ex_o = ep.tile([128, QH], BF16, tag="e",
                                   name=f"exo{pas}_{co}")
                    nc.scalar.activation(ex_o[:], lg_o[:],
                                         mybir.ActivationFunctionType.Exp,
                                         bias=mb_s[:, co:co + 1], scale=SCALE)
                    exs[co] = ex_o

            def mm2(pas, c):
                ctx_ps = ctx_tiles[pas]
                i = nmm2[0]
                nmm2[0] += 1
                first = (i % NJK == 0)
                last = (i % NJK == NJK - 1)
                for s2 in range(2):
                    nc.tensor.matmul(
                        ctx_ps[:, s2 * SR:(s2 + 1) * SR],
                        v_aug[:, c * 65:(c + 1) * 65],
                        exs[c][:, s2 * SR:(s2 + 1) * SR],
                        start=first, stop=last)

            def ctx_evac(pas):
                q0 = pas * QH
                ctx_ps = ctx_tiles[pas]
                nc.vector.tensor_copy(ctxu[0:64, q0:q0 + QH], ctx_ps[0:64, :])
                nc.vector.tensor_copy(zr[64:65, q0:q0 + QH], ctx_ps[64:65, :])
                if UP_PACKED:
                    nc.gpsimd.dma_start(out=ctxu[64:128, q0:q0 + QH],
                                        in_=ctxu[0:64, q0:q0 + QH])

            def z_recip(pas):
                # transpose Z [1,1024] -> query-major [128,8] via 8 rank-1
                # matmuls (lhsT = Z block, rhs = scalar 1.0), then reciprocal
                q0 = pas * QH
                zt_ps = PT.tile([128, 16], F32, tag="t", name=f"zt{pas}")
                for st in range(8):
                    nc.tensor.matmul(
                        zt_ps[:, st:st + 1],
                        zr[64:65, q0 + st * JC:q0 + (st + 1) * JC],
                        onesf[64:65, 0:1], start=True, stop=True)
                nc.vector.reciprocal(rzbc[:, pas * 8:pas * 8 + 8],
                                     zt_ps[:, 0:8])

            def up_tile(st):
                # st in 0..15; q-rows st*128:(st+1)*128; T8 for odd st
                hi = UP_PACKED and (st % 2 == 1)
                lh = slice(64, 128) if hi else slice(0, 64)
                osb = ob.tile([128, D], F16, tag="o", name=f"osb{st}")
                for s2 in range(2):
                    pool = PP if s2 == 0 else PT
                    up = pool.tile([128, SR], F32, tag="p" if s2 == 0 else "t",
                                   name=f"up{st}_{s2}")
                    nc.tensor.matmul(
                        up[:], ctxu[lh, st * JC:(st + 1) * JC],
                        wu_s[lh, s2 * SR:(s2 + 1) * SR],
                        start=True, stop=True)
                    nc.vector.scalar_tensor_tensor(
                        osb[:, s2 * SR:(s2 + 1) * SR], up[:],
                        rzbc[:, st:st + 1], bub_s[:, s2 * SR:(s2 + 1) * SR],
                        mybir.AluOpType.mult, mybir.AluOpType.add)
                eng = nc.sync if st < 8 else (nc.scalar if st % 2 else nc.gpsimd)
                eng.dma_start(out=out_d[st * JC:(st + 1) * JC, :], in_=osb[:])

            # ---- prologue: first ranges chase the DMAs --------------------
            kv_range(0)
            warm(2)
            q_range(0)
            for k in range(4, 8):
                xk_dma(nc.gpsimd, k, 1024, K_CAP - 1024)
            warm(2)
            q_range(1)
            warm(2)
            kv_range(1)
            v_trans(0)
            v_trans(1)
            for k in range(5, 8):
                xq_dma(nc.gpsimd, k, 1024, 1024)
            warm(2)

            # per-(pass, step) PE filler emitted *after* MM1 of that step
            fillers = {
                (0, 1): [lambda: kv_range(2), lambda: v_trans(2)],
                (0, 2): [lambda: q_range(2)],
                (0, 3): [lambda: kv_range(3), lambda: v_trans(3)],
                (0, 4): [lambda: q_range(3)],
                (0, 5): [lambda: kv_range(4), lambda: v_trans(4)],
                (1, 0): [bu_bcast],
                (1, 2): [lambda: up_tile(0), lambda: up_tile(1)],
                (1, 3): [lambda: up_tile(2), lambda: up_tile(3)],
                (1, 4): [lambda: up_tile(4), lambda: up_tile(5)],
                (1, 5): [lambda: up_tile(6)],
                (1, 6): [lambda: up_tile(7)],
            }

            steps = PAIRS + [(SOLO, None)]
            for pas in range(2):
                ctx_tiles[pas] = PC.tile([65, QH], F32, tag="c",
                                         name=f"ctx{pas}")
                for si, (ce, co) in enumerate(steps):
                    # MM2 lag-1: previous step's chunks
                    if si > 0:
                        pe, po = steps[si - 1]
                        mm2(pas, pe)
                        if po is not None:
                            mm2(pas, po)
                    elif pas == 1:
                        # cross-pass: solo chunk of pass A
                        mm2(0, SOLO)
                        ctx_evac(0)
                    mm1_exp(pas, ce, co)
                    for f in fillers.get((pas, si), []):
                        f()
                    if pas == 1 and si == 1:
                        z_recip(0)
                # drain: last step's MM2 (solo) for pass A is emitted at the
                # start of pass B; pass B's here:
                if pas == 1:
                    mm2(1, SOLO)
            ctx_evac(1)
            z_recip(1)
            for st in range(8, 16):
                up_tile(st)

    nc.compile()
    return nc


def get_graph():
    if "graph" not in _CACHE:
        _CACHE["graph"] = build_graph()
    return _CACHE["graph"]


def make_in_maps(x, attention_mask, Wd, bd, Wu, bu):
    # wd2 per k-chunk: [q(64) | q(64) | k | v | v | k]  (384 cols)
    wd2 = np.empty((128, 8 * 384), np.float16)
    for k in range(8):
        blk = Wd[k * 128:(k + 1) * 128, :].astype(np.float16)
        q_, k_, v_ = blk[:, 0:64], blk[:, 64:128], blk[:, 128:192]
        wd2[:, k * 384:(k + 1) * 384] = np.concatenate(
            [q_, q_, k_, v_, v_, k_], axis=1)
    wu2 = np.ascontiguousarray(
        np.concatenate([Wu, Wu], axis=0).astype(mybir.dt.np(mybir.dt.bfloat16)))
    bu_row = np.ascontiguousarray(bu.reshape(1, D).astype(np.float32))
    bdq2 = np.concatenate([bd[0:64], bd[0:64]]).reshape(128, 1).astype(np.float32)
    # col 0: even ranges [k-bias | v-bias]; col 1: odd ranges [v | k]
    bdkv2 = np.stack([
        np.concatenate([bd[64:128], bd[128:192]]),
        np.concatenate([bd[128:192], bd[64:128]]),
    ], axis=1).astype(np.float32)
    per_batch = []
    for b in range(B):
        idx = np.nonzero(attention_mask[b])[0]
        n = len(idx)
        assert n <= K_CAP, f"unmasked key count {n} exceeds K_CAP={K_CAP}"
        idxp = np.concatenate([idx, np.zeros(K_CAP - n, np.int64)])
        xkT = np.ascontiguousarray(
            x[b][idxp].T.astype(np.float16).reshape(8, 128, K_CAP))
        mb = np.full(K_CAP, MASKED_BIAS, np.float32)
        mb[:n] = LOGIT_SHIFT
        per_batch.append((xkT, np.ascontiguousarray(mb.reshape(NJK, 128).T)))
    in_maps = []
    for c in range(N_CORES):
        b, h = c // 2, c % 2
        xkT, mb = per_batch[b]
        xT = np.ascontiguousarray(
            x[b, h * S_LOC:(h + 1) * S_LOC].T.astype(np.float16)
            .reshape(8, 128, S_LOC))
        in_maps.append({
            "xT": xT,
            "xkT": xkT,
            "Wd2": wd2,
            "Wu2": wu2,
            "bu_row": bu_row,
            "bd_q2": bdq2,
            "bd_kv2": bdkv2,
            "maskbias": mb,
        })
    return in_maps


def kernel(x, attention_mask, Wd, bd, Wu, bu):
    from concourse import bass_utils

    x = np.asarray(x, dtype=np.float32)
    attention_mask = np.asarray(attention_mask)
    Wd = np.asarray(Wd, dtype=np.float32)
    bd = np.asarray(bd, dtype=np.float32)
    Wu = np.asarray(Wu, dtype=np.float32)
    bu = np.asarray(bu, dtype=np.float32)

    nc = get_graph()
    in_maps = make_in_maps(x, attention_mask, Wd, bd, Wu, bu)
    res = bass_utils.run_bass_kernel_spmd(nc, in_maps, list(range(N_CORES)))
    out = np.empty((B, S, D), dtype=np.float32)
    for c in range(N_CORES):
        b, h = c // 2, c % 2
        out[b, h * S_LOC:(h + 1) * S_LOC, :] = \
            res.results[c]["out"].astype(np.float32)
    return out
